# revision 36
# baseline (speedup 1.0000x reference)
"""Sparse multi-head attention (B=4, S=2048, F=512, H=8, D=64) on 8 trn2 cores.

Sharding: core c handles batch b = c % 4 and heads [hg*4, hg*4+4) with
hg = c // 4.  Zero duplicated FLOPs: QKV projection is computed per
(batch, head-group), attention per (head, batch).

Device-side layout ("scores transposed"), heads processed in PAIRS so the
projection runs M=128 matmuls (head j0 in partitions 0:64, j1 in 64:128 —
evictions stay partition-preserving and scores operands share a base
partition):
  projT[c, s] = W_pair^T @ X_b^T   (fp16 matmul; f32 psum + bias -> DVE)
  S^T [t, s]  = matmul(lhsT=Kt [d, t128], rhs=Qt [d, s512])  (fp16, f32 acc)
  E = exp(S^T)  (ACT, bf16 out; no max-subtraction: |scores| <= ~18)
  E = E * maskT (bf16 multiplicative mask, DVE 2x mode)
  Y^T_aug [65, s] += matmul(lhsT=V_aug [t128, 65], rhs=E)   (bf16)
      where V_aug has a ones column => row 64 = sum of masked exp
Host divides by the denominator and interleaves heads into the output.
The 1/sqrt(H) score scale is folded into Wq/bq on the host.
"""

import sys

for _p in ("/opt/trn_rl_repo", "/root/.axon_site/_ro/trn_rl_repo"):
    if _p not in sys.path:
        sys.path.insert(0, _p)

from contextlib import ExitStack

import ml_dtypes
import numpy as np

import concourse.bacc as bacc
import concourse.tile as tile
from concourse import bass_utils, mybir
from concourse.masks import make_identity

B, S, F, H, D = 4, 2048, 512, 8, 64
HPC = H // 2  # heads per core (4): 2 head-groups x 4 batches = 8 cores
NPAIR = HPC // 2  # head pairs per core (2)
N_CORES = 8
NF = F // 128  # 4 f-chunks of 128
NT = S // 128  # 16 t-blocks
NSP = S // 1024  # 2 query-block pairs of 1024

F32 = mybir.dt.float32
BF16 = mybir.dt.bfloat16
FP16 = mybir.dt.float16
AF = mybir.ActivationFunctionType
ALU = mybir.AluOpType


def build_nc():
    nc = bacc.Bacc(
        "TRN2", target_bir_lowering=False, debug=False, num_devices=N_CORES
    )
    xt_d = nc.dram_tensor("xt", [F, S], FP16, kind="ExternalInput").ap()
    mk_d = nc.dram_tensor("msk", [S, S], BF16, kind="ExternalInput").ap()
    # per pair m: [q_j0|q_j1|k_j0|k_j1] (256 cols)  /  [v_j0|v_j1] (128 cols)
    wqk_d = nc.dram_tensor("wqk", [F, NPAIR * 256], FP16, kind="ExternalInput").ap()
    wv_d = nc.dram_tensor("wv", [F, NPAIR * 128], FP16, kind="ExternalInput").ap()
    bias_d = nc.dram_tensor("bias", [128, 3 * NPAIR], F32, kind="ExternalInput").ap()
    yt_d = nc.dram_tensor("yt", [HPC, 65, S], F32, kind="ExternalOutput").ap()
    junk_d = nc.dram_tensor("junk", [64, 512], F32)  # warmup sink (Internal)

    with ExitStack() as ctx:
        tc = ctx.enter_context(tile.TileContext(nc))
        const = ctx.enter_context(tc.tile_pool(name="const", bufs=1))

        # identity replicated in both partition halves so transposes of
        # either head of a pair have matching operand base partitions
        ident2 = const.tile([128, 64], BF16)
        make_identity(nc, ident2[0:64, :])
        make_identity(nc, ident2[64:128, :])

        wqk_sb = const.tile([128, NF, NPAIR * 256], FP16)
        nc.sync.dma_start(wqk_sb[:], wqk_d.rearrange("(c p) n -> p c n", p=128))
        bias_sb = const.tile([128, 3 * NPAIR], F32)
        nc.sync.dma_start(bias_sb[:], bias_d)

        xt_sb = const.tile([128, NF, S], FP16)
        xt_r = xt_d.rearrange("(c p) s -> p c s", p=128)
        wv_sb = const.tile([128, NF, NPAIR * 128], FP16)
        # s-half-sliced so the first projection blocks can start after ~1MB
        for sh in range(2):
            hsl = slice(sh * (S // 2), (sh + 1) * (S // 2))
            for c in range(NF):
                nc.sync.dma_start(xt_sb[:, c, hsl], xt_r[:, c, hsl])
            if sh == 0:
                nc.sync.dma_start(
                    wv_sb[:], wv_d.rearrange("(c p) n -> p c n", p=128)
                )
        mk_sb = const.tile([128, NT, S], BF16)
        mk_r = mk_d.rearrange("(t p) s -> p t s", p=128)
        for i in range(8):
            g = NT // 8
            nc.sync.dma_start(
                mk_sb[:, i * g : (i + 1) * g, :], mk_r[:, i * g : (i + 1) * g, :]
            )

        qk_pool = ctx.enter_context(tc.tile_pool(name="qk", bufs=2))
        vt_pool = ctx.enter_context(tc.tile_pool(name="vt", bufs=2))
        v_pool = ctx.enter_context(tc.tile_pool(name="v", bufs=4))
        e_pool = ctx.enter_context(tc.tile_pool(name="e", bufs=6))
        y_pool = ctx.enter_context(tc.tile_pool(name="y", bufs=6))
        misc_ps = ctx.enter_context(tc.tile_pool(name="mps", bufs=2, space="PSUM"))
        sc_ps = ctx.enter_context(tc.tile_pool(name="sps", bufs=2, space="PSUM"))
        y_ps = ctx.enter_context(tc.tile_pool(name="yps", bufs=2, space="PSUM"))

        # --- PE warmup: junk matmuls with a full 128x128 stationary (the HAM
        # activity monitor needs the array genuinely busy) and no DMA
        # dependency, so the clock-gate opens to K=8/8 and the input DMA wait
        # is covered before the real work arrives.
        NWU = 28
        wu = const.tile([128, 512], BF16)
        nc.vector.memset(wu[:], 0.0)
        pw = sc_ps.tile([128, 512], F32, tag="s", name="pw")
        for i in range(NWU):
            nc.tensor.matmul(
                pw[:], wu[:, 0:128], wu[:], start=(i == 0), stop=(i == NWU - 1)
            )
        wu_out = const.tile([64, 512], F32)
        nc.vector.tensor_copy(wu_out[:], pw[0:64, :])
        nc.sync.dma_start(junk_d.ap(), wu_out[:])

        for m in range(NPAIR):
            # --- QKV projection for head pair (j0=2m, j1=2m+1):
            # qtP/ktP [128, S]: rows 0:64 = head j0, rows 64:128 = head j1.
            qtP = qk_pool.tile([128, S], FP16, tag="qt")
            ktP = qk_pool.tile([128, S], FP16, tag="kt")
            vtP = vt_pool.tile([128, S], BF16, tag="vt")
            for sq in range(S // 512):
                ssl = slice(sq * 512, (sq + 1) * 512)
                for out_sb, w_sb, wsl, bcol in (
                    (qtP, wqk_sb, slice(m * 256, m * 256 + 128), 3 * m),
                    (ktP, wqk_sb, slice(m * 256 + 128, (m + 1) * 256), 3 * m + 1),
                    (vtP, wv_sb, slice(m * 128, (m + 1) * 128), 3 * m + 2),
                ):
                    pp = misc_ps.tile([128, 512], F32, tag="m", name="pp")
                    for c in range(NF):
                        nc.tensor.matmul(
                            pp[:],
                            w_sb[:, c, wsl],
                            xt_sb[:, c, ssl],
                            start=(c == 0),
                            stop=(c == NF - 1),
                        )
                    # psum + per-partition bias, on DVE
                    nc.vector.tensor_scalar(
                        out_sb[:, ssl],
                        pp[:],
                        bias_sb[:, bcol : bcol + 1],
                        None,
                        op0=ALU.add,
                    )

            for half in range(2):
                j = 2 * m + half
                rsl = slice(64 * half, 64 * (half + 1))
                # --- V_aug [t, 65] per t-block: transpose V^T + ones column.
                v = v_pool.tile([128, NT, 65], BF16, tag="v")
                nc.vector.memset(v[:, :, 64:65], 1.0)
                for tb in range(NT):
                    pt = misc_ps.tile([128, 64], BF16, tag="m", name="pt")
                    nc.tensor.transpose(
                        pt[:], vtP[rsl, tb * 128 : (tb + 1) * 128], ident2[rsl, :]
                    )
                    nc.vector.tensor_copy(v[:, tb, 0:64], pt[:])

                # --- Attention over query-block pairs; accumulate over t.
                for sp in range(NSP):
                    psl = slice(sp * 1024, (sp + 1) * 1024)
                    py0 = y_ps.tile([65, 512], F32, tag="y", name="py0")
                    py1 = y_ps.tile([65, 512], F32, tag="y", name="py1")
                    for tb in range(NT):
                        ktb = ktP[rsl, tb * 128 : (tb + 1) * 128]
                        ps = sc_ps.tile([128, 1024], F32, tag="s")
                        nc.tensor.matmul(
                            ps[:, 0:512],
                            ktb,
                            qtP[rsl, sp * 1024 : sp * 1024 + 512],
                            start=True,
                            stop=True,
                        )
                        nc.tensor.matmul(
                            ps[:, 512:1024],
                            ktb,
                            qtP[rsl, sp * 1024 + 512 : (sp + 1) * 1024],
                            start=True,
                            stop=True,
                        )
                        e = e_pool.tile([128, 1024], BF16, tag="e")
                        nc.scalar.activation(e[:], ps[:], AF.Exp)
                        nc.vector.tensor_tensor(
                            e[:], e[:], mk_sb[:, tb, psl], op=ALU.mult
                        )
                        nc.tensor.matmul(
                            py0[:],
                            v[:, tb, :],
                            e[:, 0:512],
                            start=(tb == 0),
                            stop=(tb == NT - 1),
                        )
                        nc.tensor.matmul(
                            py1[:],
                            v[:, tb, :],
                            e[:, 512:1024],
                            start=(tb == 0),
                            stop=(tb == NT - 1),
                        )
                    for hh, py in ((0, py0), (1, py1)):
                        osl = slice(
                            sp * 1024 + hh * 512, sp * 1024 + (hh + 1) * 512
                        )
                        y_sb = y_pool.tile([65, 512], F32, tag="y_sb")
                        nc.vector.tensor_copy(y_sb[:], py[:])
                        nc.sync.dma_start(yt_d[j, :, osl], y_sb[:])

    nc.compile()
    return nc


_NC_CACHE = {}


def _get_nc():
    if "nc" not in _NC_CACHE:
        _NC_CACHE["nc"] = build_nc()
    return _NC_CACHE["nc"]


def make_in_maps(X, A, W, b):
    X = np.ascontiguousarray(np.asarray(X), dtype=np.float32)
    A = np.asarray(A)
    W = np.ascontiguousarray(np.asarray(W), dtype=np.float32)
    b = np.ascontiguousarray(np.asarray(b), dtype=np.float32)
    scale = np.float32(1.0 / np.sqrt(np.float32(H)))
    d = np.arange(D)

    xts = [np.ascontiguousarray(X[bb].T).astype(np.float16) for bb in range(B)]
    # multiplicative mask, transposed to [t, s], bf16 (exactly 0.0 / 1.0)
    msks = [
        np.ascontiguousarray(A[bb].T).astype(ml_dtypes.bfloat16) for bb in range(B)
    ]

    # per head-group weight/bias packs (head-pair layout)
    packs = []
    for hg in range(2):
        wqk = np.empty((F, NPAIR * 256), np.float32)
        wv = np.empty((F, NPAIR * 128), np.float32)
        bias = np.empty((128, 3 * NPAIR), np.float32)
        for m in range(NPAIR):
            for half in range(2):
                h = hg * HPC + 2 * m + half
                qc = d * 24 + h
                kc = d * 24 + 8 + h
                vc = d * 24 + 16 + h
                c0 = m * 256 + half * 64
                wqk[:, c0 : c0 + 64] = W[:, qc] * scale
                wqk[:, c0 + 128 : c0 + 192] = W[:, kc]
                wv[:, m * 128 + half * 64 : m * 128 + (half + 1) * 64] = W[:, vc]
                rsl = slice(64 * half, 64 * (half + 1))
                bias[rsl, 3 * m] = b[qc] * scale
                bias[rsl, 3 * m + 1] = b[kc]
                bias[rsl, 3 * m + 2] = b[vc]
        packs.append((wqk.astype(np.float16), wv.astype(np.float16), bias))

    in_maps = []
    for c in range(N_CORES):
        bb = c % B
        hg = c // B
        wqk, wv, bias = packs[hg]
        in_maps.append(
            {
                "xt": xts[bb],
                "msk": msks[bb],
                "wqk": wqk,
                "wv": wv,
                "bias": bias,
            }
        )
    return in_maps


def assemble_output(results):
    Y = np.empty((B, S, D * H), np.float32)
    Yv = Y.reshape(B, S, D, H)
    for c in range(N_CORES):
        bb = c % B
        hg = c // B
        yt = results[c]["yt"]  # [HPC, 65, S]
        for j in range(HPC):
            h = hg * HPC + j
            Yv[bb, :, :, h] = (yt[j, 0:64, :] / yt[j, 64:65, :]).T
    return Y


def kernel(X, A, W, b):
    nc = _get_nc()
    in_maps = make_in_maps(X, A, W, b)
    res = bass_utils.run_bass_kernel_spmd(
        nc, in_maps, core_ids=list(range(N_CORES))
    ).results
    return assemble_output(res)
